# revision 22
# baseline (speedup 1.0000x reference)
"""Distributed Trainium2 (Bass/Tile) kernel for causal *linear* attention
(unnormalized tril(q k^T) attention + state read/update) with qkv/out
projections, head-sharded over 8 NeuronCores.

Math identity used: with no softmax,
    out[t] = sum_{s<=t} (q_t . k_s) v_s + q_t @ state
           = q_t @ (state + sum_{s<=t} k_s v_s^T)
so attention is computed as a chunked scan (512-wide t-supers, 128-wide
s-blocks) instead of the O(T^2) dense form.

Distribution (8 cores): heads tensor-parallel (2 heads/core), lin_qkv
column-parallel, attention fully local per head; then one AllToAll turns the
head-sharded attention output into a sequence-sharded one, and each core runs
the out-projection for its own 512-row slice of the sequence (y computed
feature-major as yT = w_out^T @ out). Host only slices/concats/transposes.
"""

import os

import numpy as np

# ---------------------------------------------------------------- constants
T_FULL = 4096
D = 1024
H = 16
DH = 64
NCORES = 8
HPC = H // NCORES            # heads per core = 2
SUP = 512                    # t-super width
BLK = 128                    # s-block width
KT = D // 128                # k-tiles over embedding dim = 8
GW = HPC * DH                # per-group width of the qkv slice = 128

# matmul compute mode: "f32r" (fast fp32), "f32" (exact, 4x slower), "bf16"
MM_MODE = os.environ.get("KERNEL_MM_MODE", "bf16")


def build_nc(t_len=T_FULL, mm_mode=MM_MODE, num_cores=NCORES, sup=SUP):
    import concourse.bass as bass  # noqa: F401
    import concourse.mybir as mybir
    import concourse.tile as tile
    from concourse import bacc
    from concourse.masks import make_identity, make_upper_triangular

    f32 = mybir.dt.float32
    if mm_mode == "bf16":
        MDT = mybir.dt.bfloat16
    elif mm_mode == "f32r":
        # native float32r end-to-end: DMA keeps it, DVE writes perform the
        # fp32r rounding the BIR verifier demands of matmult inputs
        MDT = mybir.dt.float32r
    else:
        MDT = f32

    def mm(ap):
        return ap

    def mmf(ap):
        # fp32r matmults may not write PSUM at base partition != 0; for the
        # tiny S-update matmuls (N=64: f32 and f32r cost the same 4 cyc/row)
        # run in plain f32 instead
        return ap.bitcast(f32) if mm_mode == "f32r" else ap

    nsup = t_len // sup
    nblk = sup // BLK
    ts_len = t_len // num_cores   # per-core output sequence slice
    feat = num_cores * BLK        # gathered attention-feature dim (=D at 8 cores)
    # phase-1 supers map 1:1 onto a2a shards (t-chunks of the out-projection)
    assert ts_len == sup and nsup == num_cores

    nc = bacc.Bacc(
        "TRN2",
        target_bir_lowering=False,
        debug=False,
        enable_asserts=False,
        num_devices=num_cores,
    )

    xt_d = nc.dram_tensor("xt", [D, t_len], MDT, kind="ExternalInput")
    wqkv_d = nc.dram_tensor("wqkv", [D, 3 * GW], MDT, kind="ExternalInput")
    bqkv_d = nc.dram_tensor("bqkv", [3, GW], f32, kind="ExternalInput")
    st_d = nc.dram_tensor("state0", [HPC * DH, DH], f32, kind="ExternalInput")
    wout_d = nc.dram_tensor("wout", [feat, D], MDT, kind="ExternalInput")
    bout_d = nc.dram_tensor("bout", [KT, 128], f32, kind="ExternalInput")
    yt_d = nc.dram_tensor("yt", [D, ts_len], f32, kind="ExternalOutput")
    ns_d = nc.dram_tensor("ns", [HPC * DH, DH], f32, kind="ExternalOutput")

    with tile.TileContext(nc) as tc:
        with (
            tc.tile_pool(name="consts", bufs=1) as consts,
            tc.tile_pool(name="wpool", bufs=1) as wpool,
            tc.tile_pool(name="xtp", bufs=(4 if mm_mode == "bf16" else 3)) as xtp,
            tc.tile_pool(name="qkvp", bufs=2) as qkvp,
            tc.tile_pool(name="natp", bufs=2) as natp,
            tc.tile_pool(name="attp", bufs=4) as attp,
            tc.tile_pool(name="outp", bufs=2) as outp,
            tc.tile_pool(name="miscp", bufs=2) as miscp,
            tc.tile_pool(name="agp", bufs=1) as agp,
            tc.tile_pool(name="pp_ps", bufs=2, space="PSUM") as pp_ps,
            tc.tile_pool(name="at_ps", bufs=2, space="PSUM") as at_ps,
            tc.tile_pool(name="tr_ps", bufs=1, space="PSUM") as tr_ps,
            tc.tile_pool(name="ot_ps", bufs=1, space="PSUM") as ot_ps,
            tc.tile_pool(name="s_ps", bufs=1, space="PSUM") as s_ps,
            tc.tile_pool(name="dram", bufs=1, space="DRAM") as dram,
        ):
            # ---------------- constants / weights
            # PE transposes run in plain f32 when MDT is float32r (memset/
            # affine_select can't write fp32r, and transpose-mode is exact)
            TDT = f32 if mm_mode == "f32r" else MDT

            def td(ap):
                return ap.bitcast(TDT) if TDT is not MDT else ap

            # qkv weights + first x super first: they gate the first matmul
            wq_sb = wpool.tile([128, KT * 3 * GW], MDT, name="wq_sb")

            ident = consts.tile([128, 128], TDT, name="ident")
            make_identity(nc, ident[:])
            mask = consts.tile([BLK, sup], f32, name="mask")
            make_upper_triangular(nc, mask[:, :BLK], val=1.0, diag=True)
            if sup > BLK:
                nc.gpsimd.memset(mask[:, BLK:], 1.0)

            state_sb = consts.tile([HPC * DH, DH], f32, name="state_sb")
            nc.sync.dma_start(out=state_sb[:], in_=st_d[:])
            bq_sb = consts.tile([GW, 3], f32, name="bq_sb")
            for g in range(3):
                nc.sync.dma_start(out=bq_sb[:, g : g + 1], in_=bqkv_d[g : g + 1, :])
            bo_sb = consts.tile([128, KT], f32, name="bo_sb")
            for g in range(KT):
                nc.sync.dma_start(out=bo_sb[:, g : g + 1], in_=bout_d[g : g + 1, :])

            # a2a buffers: (num_cores * BLK, ts_len); shard j = t-chunk j,
            # feature rows (2 heads x 64) within the 128-row block.
            a2a_in = dram.tile([num_cores * BLK, ts_len], MDT, name="a2a_in")
            a2a_out = dram.tile([num_cores * BLK, ts_len], MDT, name="a2a_out")
            # last super's shard travels via a small AllGather instead (the
            # AllToAll is issued one super early to overlap the last super)
            ag2_in = dram.tile([BLK, ts_len], MDT, name="ag2_in")
            ag2_out = dram.tile(
                [num_cores * BLK, ts_len], MDT, name="ag2_out", addr_space="Shared"
            )
            zero_sb = consts.tile([BLK, ts_len], MDT, name="zero_sb")
            nc.gpsimd.memset(td(zero_sb[:]), 0.0)
            # a2a shard for the last core is dummy (real data goes via AG);
            # keep it finite
            nc.gpsimd.dma_start(
                out=a2a_in[BLK * (num_cores - 1) : BLK * num_cores, :], in_=zero_sb[:]
            )

            # running state accumulator (psum, fp32), heads stacked on partitions
            s_acc = s_ps.tile([HPC * DH, DH], f32, name="s_acc", tag="sacc")

            # ---------------- phase 1: qkv proj + chunked linear attention
            for tau in range(nsup):
                tsl = slice(sup * tau, sup * (tau + 1))
                xt_sb = xtp.tile([128, KT * sup], MDT, name="xt_sb", tag="xt")
                for k in range(KT):
                    if tau == 0:
                        # pairwise with xt k so proj MM k is gated by only
                        # 2(k+1) queued DMAs
                        nc.sync.dma_start(
                            out=wq_sb[:, 3 * GW * k : 3 * GW * (k + 1)],
                            in_=wqkv_d[128 * k : 128 * (k + 1), :],
                        )
                    nc.sync.dma_start(
                        out=xt_sb[:, sup * k : sup * (k + 1)],
                        in_=xt_d[128 * k : 128 * (k + 1), tsl],
                    )

                # qkvT projection, feature-major: group g in (q, k, v)
                grp_sb = []
                for g in range(3):
                    ps = pp_ps.tile([128, sup], f32, name=f"proj_ps{g}", tag="pp")
                    for k in range(KT):
                        nc.tensor.matmul(
                            out=ps[:],
                            lhsT=mm(wq_sb[:, 3 * GW * k + GW * g : 3 * GW * k + GW * (g + 1)]),
                            rhs=mm(xt_sb[:, sup * k : sup * (k + 1)]),
                            start=(k == 0),
                            stop=(k == KT - 1),
                        )
                    sb = qkvp.tile([128, sup], MDT, name=f"g{g}T_sb", tag=f"g{g}T")
                    nc.vector.tensor_scalar_add(sb[:], ps[:], bq_sb[:, g : g + 1])
                    grp_sb.append(sb)
                qT_sb, kT_sb, vT_sb = grp_sb

                # transpose k, v to t-major (s on partitions) via PE
                k_nat = natp.tile([128, sup], MDT, name="k_nat", tag="knat")
                v_nat = natp.tile([128, sup], MDT, name="v_nat", tag="vnat")
                for srct, dst in ((kT_sb, k_nat), (vT_sb, v_nat)):
                    for j in range(nblk):
                        bsl = slice(BLK * j, BLK * (j + 1))
                        tp = tr_ps.tile([128, BLK], TDT, name="tp_ps", tag="tp")
                        nc.tensor.transpose(tp[:], td(srct[:, bsl]), ident[:])
                        nc.vector.tensor_copy(dst[:, bsl], tp[:])

                # snapshot of state for this super (state_init + prefix sums)
                s_sb = miscp.tile([HPC * DH, DH], MDT, name="s_sb", tag="ssb")
                for h in range(HPC):
                    hsl = slice(DH * h, DH * (h + 1))
                    if tau == 0:
                        nc.vector.tensor_copy(s_sb[hsl, :], state_sb[hsl, :])
                    else:
                        nc.vector.tensor_add(s_sb[hsl, :], s_acc[hsl, :], state_sb[hsl, :])

                out_ps_h = [
                    ot_ps.tile([DH, sup], f32, name=f"out_ps{h}", tag=f"ops{h}")
                    for h in range(HPC)
                ]
                for h in range(HPC):
                    hsl = slice(DH * h, DH * (h + 1))
                    out_ps = out_ps_h[h]
                    # state contribution covers all t of the super
                    nc.tensor.matmul(
                        out=out_ps[:, :],
                        lhsT=mm(s_sb[hsl, :]),
                        rhs=mm(qT_sb[hsl, :]),
                        start=True,
                        stop=False,
                    )
                    for j in range(nblk):
                        bsl = slice(BLK * j, BLK * (j + 1))
                        used = sup - BLK * j       # out columns [BLK*j, sup)
                        if j < nblk - 1 or nblk == 1 or mm_mode != "f32r":
                            width, rstart, dg0 = used, BLK * j, 0
                        else:
                            # widen N to 256 so f32r stays on the 1-cyc path
                            width, rstart, dg0 = 2 * BLK, sup - 2 * BLK, BLK
                        aps = at_ps.tile([128, sup], f32, name="aps", tag="aps")
                        nc.tensor.matmul(
                            out=aps[:, :width],
                            lhsT=mm(kT_sb[hsl, bsl]),
                            rhs=mm(qT_sb[hsl, rstart:sup]),
                            start=True,
                            stop=True,
                        )
                        asb = attp.tile([128, sup], MDT, name="asb", tag="asb")
                        # one op: triu mask on the diagonal 128 block, pass-
                        # through (x1.0) beyond it
                        nc.vector.tensor_mul(
                            asb[:, dg0 : dg0 + used],
                            aps[:, dg0 : dg0 + used],
                            mask[:, :used],
                        )
                        nc.tensor.matmul(
                            out=out_ps_h[h][:, BLK * j : sup],
                            lhsT=mm(v_nat[:, BLK * j + DH * h : BLK * j + DH * (h + 1)]),
                            rhs=mm(asb[:, dg0 : dg0 + used]),
                            start=False,
                            stop=(j == nblk - 1),
                        )
                        # state update: S += k_blk^T v_blk (one psum group over
                        # the whole scan; reads interleave, so skip group check)
                        nc.tensor.matmul(
                            out=s_acc[hsl, :],
                            lhsT=mmf(k_nat[:, BLK * j + DH * h : BLK * j + DH * (h + 1)]),
                            rhs=mmf(v_nat[:, BLK * j + DH * h : BLK * j + DH * (h + 1)]),
                            start=(tau == 0 and j == 0),
                            stop=(tau == nsup - 1 and j == nblk - 1),
                            skip_group_check=True,
                        )

                if True:
                    for h in range(HPC):
                        o_sb = outp.tile([DH, sup], MDT, name=f"o_sb{h}", tag=f"osb{h}")
                        nc.vector.tensor_copy(o_sb[:], out_ps_h[h][:])
                        if tau < nsup - 1:
                            dst = a2a_in[BLK * tau + DH * h : BLK * tau + DH * (h + 1), :]
                        else:
                            dst = ag2_in[DH * h : DH * (h + 1), :]
                        nc.gpsimd.dma_start(out=dst, in_=o_sb[:])
                if tau == nsup - 2:
                    # all destinations except the last core are final: launch
                    # the AllToAll now so it overlaps the last super's compute
                    nc.gpsimd.collective_compute(
                        "AllToAll",
                        mybir.AluOpType.bypass,
                        replica_groups=[list(range(num_cores))],
                        ins=[a2a_in.opt()],
                        outs=[a2a_out.opt()],
                    )

            # out-proj weights: only needed after the collective; loading here
            # overlaps the DMAs with late-phase-1 compute
            wo_sb = wpool.tile([128, num_cores * D], MDT, name="wo_sb")
            for k in range(num_cores):
                nc.sync.dma_start(
                    out=wo_sb[:, D * k : D * (k + 1)],
                    in_=wout_d[128 * k : 128 * (k + 1), :],
                )

            # ---------------- phase 2: gather last shard + out-projection
            nc.gpsimd.collective_compute(
                "AllGather",
                mybir.AluOpType.bypass,
                replica_groups=[list(range(num_cores))],
                ins=[ag2_in.opt()],
                outs=[ag2_out.opt()],
            )
            ag_sb = agp.tile([128, num_cores * ts_len], MDT, name="ag_sb")
            for k in range(num_cores):
                nc.sync.dma_start(
                    out=ag_sb[:, ts_len * k : ts_len * (k + 1)],
                    in_=a2a_out[128 * k : 128 * (k + 1), :],
                )
            # on the last core the AllToAll result is dummy - overwrite from
            # the AllGather (predicated DMAs; skipped elsewhere)
            is_last = nc.sync.partition_id() == (num_cores - 1)
            for k in range(num_cores):
                nc.sync.dma_start(
                    out=ag_sb[:, ts_len * k : ts_len * (k + 1)],
                    in_=ag2_out[128 * k : 128 * (k + 1), :],
                    cond=is_last,
                )
            for m in range(KT):
                yps = pp_ps.tile([128, ts_len], f32, name="y_ps", tag="pp")
                for k in range(num_cores):
                    nc.tensor.matmul(
                        out=yps[:],
                        lhsT=mm(wo_sb[:, D * k + 128 * m : D * k + 128 * (m + 1)]),
                        rhs=mm(ag_sb[:, ts_len * k : ts_len * (k + 1)]),
                        start=(k == 0),
                        stop=(k == KT - 1),
                    )
                ysb = miscp.tile([128, ts_len], f32, name="ysb", tag="ysb")
                nc.vector.tensor_scalar_add(ysb[:], yps[:], bo_sb[:, m : m + 1])
                nc.gpsimd.dma_start(out=yt_d[128 * m : 128 * (m + 1), :], in_=ysb[:])

            # ---------------- new state output
            ns_sb = miscp.tile([HPC * DH, DH], f32, name="ns_sb", tag="nssb")
            nc.vector.tensor_add(ns_sb[:], s_acc[:], state_sb[:])
            nc.gpsimd.dma_start(out=ns_d[:], in_=ns_sb[:])

    nc.compile()
    return nc


def shard_inputs(state, x, w_qkv, b_qkv, w_out, b_out, mm_mode=MM_MODE,
                 num_cores=NCORES):
    """Host-side sharding/marshaling. Returns in_maps (one dict per core)."""
    if mm_mode == "bf16":
        import ml_dtypes
        mdt = ml_dtypes.bfloat16
    else:
        mdt = np.float32

    state = np.asarray(state, np.float32)
    x = np.asarray(x, np.float32)
    w_qkv = np.asarray(w_qkv, np.float32)
    b_qkv = np.asarray(b_qkv, np.float32)
    w_out = np.asarray(w_out, np.float32)
    b_out = np.asarray(b_out, np.float32)

    xt = np.ascontiguousarray(x.T).astype(mdt)          # (D, T)
    wo = w_out.astype(mdt)                              # (D, D)
    bo = b_out.reshape(KT, 128)                         # (8, 128)

    d_embd = x.shape[1]
    in_maps = []
    for core in range(num_cores):
        h0 = HPC * core
        cols = np.concatenate(
            [
                np.arange(g * d_embd + (h0 + hh) * DH, g * d_embd + (h0 + hh) * DH + DH)
                for g in range(3)
                for hh in range(HPC)
            ]
        )
        in_maps.append(
            {
                "xt": xt,
                "wqkv": np.ascontiguousarray(w_qkv[:, cols]).astype(mdt),
                "bqkv": np.ascontiguousarray(b_qkv[cols].reshape(3, HPC * DH)),
                "state0": np.ascontiguousarray(
                    state[h0 : h0 + HPC].reshape(HPC * DH, DH)
                ),
                "wout": wo,
                "bout": np.ascontiguousarray(bo),
            }
        )
    return in_maps


def assemble_outputs(results, t_len=T_FULL, num_cores=NCORES):
    ts_len = t_len // num_cores
    y = np.empty((t_len, D), np.float32)
    new_state = np.empty((H, DH, DH), np.float32)
    for core in range(num_cores):
        y[ts_len * core : ts_len * (core + 1), :] = results[core]["yt"].T
        new_state[HPC * core : HPC * (core + 1)] = results[core]["ns"].reshape(
            HPC, DH, DH
        )
    return new_state, y


_NC_CACHE = {}
LAST_RESULTS = None


def kernel(state, x, w_qkv, b_qkv, w_out, b_out):
    global LAST_RESULTS
    from concourse import bass_utils

    key = (MM_MODE,)
    if key not in _NC_CACHE:
        _NC_CACHE[key] = build_nc(T_FULL, MM_MODE, NCORES)
    nc = _NC_CACHE[key]

    in_maps = shard_inputs(state, x, w_qkv, b_qkv, w_out, b_out, MM_MODE, NCORES)
    res = bass_utils.run_bass_kernel_spmd(
        nc, in_maps, core_ids=list(range(NCORES))
    )
    LAST_RESULTS = res
    return assemble_outputs(res.results, T_FULL, NCORES)


# revision 23
# speedup vs baseline: 1.1341x; 1.1341x over previous
"""Distributed Trainium2 (Bass/Tile) kernel for causal *linear* attention
(unnormalized tril(q k^T) attention + state read/update) with qkv/out
projections, head-sharded over 8 NeuronCores.

Math identity used: with no softmax,
    out[t] = sum_{s<=t} (q_t . k_s) v_s + q_t @ state
           = q_t @ (state + sum_{s<=t} k_s v_s^T)
so attention is computed as a chunked scan (512-wide t-supers, 128-wide
s-blocks) instead of the O(T^2) dense form.

Distribution (8 cores): heads tensor-parallel (2 heads/core), lin_qkv
column-parallel, attention fully local per head; then one AllToAll turns the
head-sharded attention output into a sequence-sharded one, and each core runs
the out-projection for its own 512-row slice of the sequence (y computed
feature-major as yT = w_out^T @ out). Host only slices/concats/transposes.
"""

import os

import numpy as np

# ---------------------------------------------------------------- constants
T_FULL = 4096
D = 1024
H = 16
DH = 64
NCORES = 8
HPC = H // NCORES            # heads per core = 2
SUP = 512                    # t-super width
BLK = 128                    # s-block width
KT = D // 128                # k-tiles over embedding dim = 8
GW = HPC * DH                # per-group width of the qkv slice = 128

# matmul compute mode: "f32r" (fast fp32), "f32" (exact, 4x slower), "bf16"
MM_MODE = os.environ.get("KERNEL_MM_MODE", "bf16")


def build_nc(t_len=T_FULL, mm_mode=MM_MODE, num_cores=NCORES, sup=SUP):
    import concourse.bass as bass  # noqa: F401
    import concourse.mybir as mybir
    import concourse.tile as tile
    from concourse import bacc
    from concourse.masks import make_identity, make_upper_triangular

    f32 = mybir.dt.float32
    if mm_mode == "bf16":
        MDT = mybir.dt.bfloat16
    elif mm_mode == "f32r":
        # native float32r end-to-end: DMA keeps it, DVE writes perform the
        # fp32r rounding the BIR verifier demands of matmult inputs
        MDT = mybir.dt.float32r
    else:
        MDT = f32

    def mm(ap):
        return ap

    def mmf(ap):
        # fp32r matmults may not write PSUM at base partition != 0; for the
        # tiny S-update matmuls (N=64: f32 and f32r cost the same 4 cyc/row)
        # run in plain f32 instead
        return ap.bitcast(f32) if mm_mode == "f32r" else ap

    nsup = t_len // sup
    nblk = sup // BLK
    ts_len = t_len // num_cores   # per-core output sequence slice
    feat = num_cores * BLK        # gathered attention-feature dim (=D at 8 cores)
    # phase-1 supers map 1:1 onto a2a shards (t-chunks of the out-projection)
    assert ts_len == sup and nsup == num_cores

    nc = bacc.Bacc(
        "TRN2",
        target_bir_lowering=False,
        debug=False,
        enable_asserts=False,
        num_devices=num_cores,
    )

    xt_d = nc.dram_tensor("xt", [D, t_len], MDT, kind="ExternalInput")
    wqkv_d = nc.dram_tensor("wqkv", [D, 3 * GW], MDT, kind="ExternalInput")
    bqkv_d = nc.dram_tensor("bqkv", [3, GW], f32, kind="ExternalInput")
    st_d = nc.dram_tensor("state0", [HPC * DH, DH], f32, kind="ExternalInput")
    wout_d = nc.dram_tensor("wout", [feat, D], MDT, kind="ExternalInput")
    bout_d = nc.dram_tensor("bout", [KT, 128], f32, kind="ExternalInput")
    yt_d = nc.dram_tensor("yt", [D, ts_len], f32, kind="ExternalOutput")
    ns_d = nc.dram_tensor("ns", [HPC * DH, DH], f32, kind="ExternalOutput")

    with tile.TileContext(nc) as tc:
        with (
            tc.tile_pool(name="consts", bufs=1) as consts,
            tc.tile_pool(name="wpool", bufs=1) as wpool,
            tc.tile_pool(name="xtp", bufs=(4 if mm_mode == "bf16" else 3)) as xtp,
            tc.tile_pool(name="qkvp", bufs=2) as qkvp,
            tc.tile_pool(name="natp", bufs=2) as natp,
            tc.tile_pool(name="attp", bufs=4) as attp,
            tc.tile_pool(name="outp", bufs=2) as outp,
            tc.tile_pool(name="miscp", bufs=2) as miscp,
            tc.tile_pool(name="agp", bufs=1) as agp,
            tc.tile_pool(name="pp_ps", bufs=2, space="PSUM") as pp_ps,
            tc.tile_pool(name="at_ps", bufs=2, space="PSUM") as at_ps,
            tc.tile_pool(name="tr_ps", bufs=1, space="PSUM") as tr_ps,
            tc.tile_pool(name="ot_ps", bufs=1, space="PSUM") as ot_ps,
            tc.tile_pool(name="s_ps", bufs=1, space="PSUM") as s_ps,
            tc.tile_pool(name="dram", bufs=1, space="DRAM") as dram,
        ):
            # ---------------- constants / weights
            # PE transposes run in plain f32 when MDT is float32r (memset/
            # affine_select can't write fp32r, and transpose-mode is exact)
            TDT = f32 if mm_mode == "f32r" else MDT

            def td(ap):
                return ap.bitcast(TDT) if TDT is not MDT else ap

            # qkv weights + first x super first: they gate the first matmul
            wq_sb = wpool.tile([128, KT * 3 * GW], MDT, name="wq_sb")

            ident = consts.tile([128, 128], TDT, name="ident")
            make_identity(nc, ident[:])
            mask = consts.tile([BLK, sup], f32, name="mask")
            make_upper_triangular(nc, mask[:, :BLK], val=1.0, diag=True)
            if sup > BLK:
                nc.gpsimd.memset(mask[:, BLK:], 1.0)

            state_sb = consts.tile([HPC * DH, DH], f32, name="state_sb")
            nc.sync.dma_start(out=state_sb[:], in_=st_d[:])
            bq_sb = consts.tile([GW, 3], f32, name="bq_sb")
            for g in range(3):
                nc.sync.dma_start(out=bq_sb[:, g : g + 1], in_=bqkv_d[g : g + 1, :])
            bo_sb = consts.tile([128, KT], f32, name="bo_sb")
            for g in range(KT):
                nc.sync.dma_start(out=bo_sb[:, g : g + 1], in_=bout_d[g : g + 1, :])

            # a2a buffers: (num_cores * BLK, ts_len); shard j = t-chunk j,
            # feature rows (2 heads x 64) within the 128-row block.
            a2a_in = dram.tile([num_cores * BLK, ts_len], MDT, name="a2a_in")
            a2a_out = dram.tile([num_cores * BLK, ts_len], MDT, name="a2a_out")
            # last super's shard travels via a small AllGather instead (the
            # AllToAll is issued one super early to overlap the last super)
            ag2_in = dram.tile([BLK, ts_len], MDT, name="ag2_in")
            ag2_out = dram.tile(
                [num_cores * BLK, ts_len], MDT, name="ag2_out", addr_space="Shared"
            )
            zero_sb = consts.tile([BLK, ts_len], MDT, name="zero_sb")
            nc.gpsimd.memset(td(zero_sb[:]), 0.0)
            # a2a shard for the last core is dummy (real data goes via AG);
            # keep it finite
            nc.gpsimd.dma_start(
                out=a2a_in[BLK * (num_cores - 1) : BLK * num_cores, :], in_=zero_sb[:]
            )

            # running state accumulator (psum, fp32), heads stacked on partitions
            s_acc = s_ps.tile([HPC * DH, DH], f32, name="s_acc", tag="sacc")

            # ---------------- phase 1: qkv proj + chunked linear attention
            for tau in range(nsup):
                tsl = slice(sup * tau, sup * (tau + 1))
                xt_sb = xtp.tile([128, KT * sup], MDT, name="xt_sb", tag="xt")
                for k in range(KT):
                    if tau == 0:
                        # pairwise with xt k so proj MM k is gated by only
                        # 2(k+1) queued DMAs
                        nc.sync.dma_start(
                            out=wq_sb[:, 3 * GW * k : 3 * GW * (k + 1)],
                            in_=wqkv_d[128 * k : 128 * (k + 1), :],
                        )
                    nc.sync.dma_start(
                        out=xt_sb[:, sup * k : sup * (k + 1)],
                        in_=xt_d[128 * k : 128 * (k + 1), tsl],
                    )

                # qkvT projection, feature-major: group g in (q, k, v)
                grp_sb = []
                for g in range(3):
                    ps = pp_ps.tile([128, sup], f32, name=f"proj_ps{g}", tag="pp")
                    for k in range(KT):
                        nc.tensor.matmul(
                            out=ps[:],
                            lhsT=mm(wq_sb[:, 3 * GW * k + GW * g : 3 * GW * k + GW * (g + 1)]),
                            rhs=mm(xt_sb[:, sup * k : sup * (k + 1)]),
                            start=(k == 0),
                            stop=(k == KT - 1),
                        )
                    sb = qkvp.tile([128, sup], MDT, name=f"g{g}T_sb", tag=f"g{g}T")
                    nc.vector.tensor_scalar_add(sb[:], ps[:], bq_sb[:, g : g + 1])
                    grp_sb.append(sb)
                qT_sb, kT_sb, vT_sb = grp_sb

                # transpose k, v to t-major (s on partitions) via PE
                k_nat = natp.tile([128, sup], MDT, name="k_nat", tag="knat")
                v_nat = natp.tile([128, sup], MDT, name="v_nat", tag="vnat")
                for srct, dst in ((kT_sb, k_nat), (vT_sb, v_nat)):
                    for j in range(nblk):
                        bsl = slice(BLK * j, BLK * (j + 1))
                        tp = tr_ps.tile([128, BLK], TDT, name="tp_ps", tag="tp")
                        nc.tensor.transpose(tp[:], td(srct[:, bsl]), ident[:])
                        nc.vector.tensor_copy(dst[:, bsl], tp[:])

                # snapshot of state for this super (state_init + prefix sums)
                s_sb = miscp.tile([HPC * DH, DH], MDT, name="s_sb", tag="ssb")
                for h in range(HPC):
                    hsl = slice(DH * h, DH * (h + 1))
                    if tau == 0:
                        nc.vector.tensor_copy(s_sb[hsl, :], state_sb[hsl, :])
                    else:
                        nc.vector.tensor_add(s_sb[hsl, :], s_acc[hsl, :], state_sb[hsl, :])

                out_ps_h = [
                    ot_ps.tile([DH, sup], f32, name=f"out_ps{h}", tag=f"ops{h}")
                    for h in range(HPC)
                ]
                # the two heads' K=64 matmuls sit at row-group bases 0/64;
                # emitting each pair back-to-back lets the PE run them
                # concurrently in disjoint 32x32 subarrays
                for h in range(HPC):
                    hsl = slice(DH * h, DH * (h + 1))
                    # state contribution covers all t of the super
                    nc.tensor.matmul(
                        out=out_ps_h[h][:, :],
                        lhsT=mm(s_sb[hsl, :]),
                        rhs=mm(qT_sb[hsl, :]),
                        start=True,
                        stop=False,
                    )
                for j in range(nblk):
                    bsl = slice(BLK * j, BLK * (j + 1))
                    used = sup - BLK * j           # out columns [BLK*j, sup)
                    if j < nblk - 1 or nblk == 1 or mm_mode != "f32r":
                        width, rstart, dg0 = used, BLK * j, 0
                    else:
                        # widen N to 256 so f32r stays on the 1-cyc path
                        width, rstart, dg0 = 2 * BLK, sup - 2 * BLK, BLK
                    aps_h, asb_h = [], []
                    for h in range(HPC):
                        hsl = slice(DH * h, DH * (h + 1))
                        aps = at_ps.tile([128, sup], f32, name=f"aps{h}", tag="aps")
                        nc.tensor.matmul(
                            out=aps[:, :width],
                            lhsT=mm(kT_sb[hsl, bsl]),
                            rhs=mm(qT_sb[hsl, rstart:sup]),
                            start=True,
                            stop=True,
                        )
                        aps_h.append(aps)
                    for h in range(HPC):
                        asb = attp.tile([128, sup], MDT, name=f"asb{h}", tag="asb")
                        # one op: triu mask on the diagonal 128 block, pass-
                        # through (x1.0) beyond it
                        nc.vector.tensor_mul(
                            asb[:, dg0 : dg0 + used],
                            aps_h[h][:, dg0 : dg0 + used],
                            mask[:, :used],
                        )
                        asb_h.append(asb)
                    for h in range(HPC):
                        nc.tensor.matmul(
                            out=out_ps_h[h][:, BLK * j : sup],
                            lhsT=mm(v_nat[:, BLK * j + DH * h : BLK * j + DH * (h + 1)]),
                            rhs=mm(asb_h[h][:, dg0 : dg0 + used]),
                            start=False,
                            stop=(j == nblk - 1),
                        )
                    for h in range(HPC):
                        hsl = slice(DH * h, DH * (h + 1))
                        # state update: S += k_blk^T v_blk (one psum group over
                        # the whole scan; reads interleave, so skip group check)
                        nc.tensor.matmul(
                            out=s_acc[hsl, :],
                            lhsT=mmf(k_nat[:, BLK * j + DH * h : BLK * j + DH * (h + 1)]),
                            rhs=mmf(v_nat[:, BLK * j + DH * h : BLK * j + DH * (h + 1)]),
                            start=(tau == 0 and j == 0),
                            stop=(tau == nsup - 1 and j == nblk - 1),
                            skip_group_check=True,
                        )

                if True:
                    for h in range(HPC):
                        o_sb = outp.tile([DH, sup], MDT, name=f"o_sb{h}", tag=f"osb{h}")
                        nc.vector.tensor_copy(o_sb[:], out_ps_h[h][:])
                        if tau < nsup - 1:
                            dst = a2a_in[BLK * tau + DH * h : BLK * tau + DH * (h + 1), :]
                        else:
                            dst = ag2_in[DH * h : DH * (h + 1), :]
                        nc.gpsimd.dma_start(out=dst, in_=o_sb[:])
                if tau == nsup - 2:
                    # all destinations except the last core are final: launch
                    # the AllToAll now so it overlaps the last super's compute
                    nc.gpsimd.collective_compute(
                        "AllToAll",
                        mybir.AluOpType.bypass,
                        replica_groups=[list(range(num_cores))],
                        ins=[a2a_in.opt()],
                        outs=[a2a_out.opt()],
                    )

            # out-proj weights: only needed after the collective; loading here
            # overlaps the DMAs with late-phase-1 compute
            wo_sb = wpool.tile([128, num_cores * D], MDT, name="wo_sb")
            for k in range(num_cores):
                nc.sync.dma_start(
                    out=wo_sb[:, D * k : D * (k + 1)],
                    in_=wout_d[128 * k : 128 * (k + 1), :],
                )

            # ---------------- phase 2: gather last shard + out-projection
            nc.gpsimd.collective_compute(
                "AllGather",
                mybir.AluOpType.bypass,
                replica_groups=[list(range(num_cores))],
                ins=[ag2_in.opt()],
                outs=[ag2_out.opt()],
            )
            ag_sb = agp.tile([128, num_cores * ts_len], MDT, name="ag_sb")
            for k in range(num_cores):
                nc.sync.dma_start(
                    out=ag_sb[:, ts_len * k : ts_len * (k + 1)],
                    in_=a2a_out[128 * k : 128 * (k + 1), :],
                )
            # on the last core the AllToAll result is dummy - overwrite from
            # the AllGather (predicated DMAs; skipped elsewhere)
            is_last = nc.sync.partition_id() == (num_cores - 1)
            for k in range(num_cores):
                nc.sync.dma_start(
                    out=ag_sb[:, ts_len * k : ts_len * (k + 1)],
                    in_=ag2_out[128 * k : 128 * (k + 1), :],
                    cond=is_last,
                )
            for m in range(KT):
                yps = pp_ps.tile([128, ts_len], f32, name="y_ps", tag="pp")
                for k in range(num_cores):
                    nc.tensor.matmul(
                        out=yps[:],
                        lhsT=mm(wo_sb[:, D * k + 128 * m : D * k + 128 * (m + 1)]),
                        rhs=mm(ag_sb[:, ts_len * k : ts_len * (k + 1)]),
                        start=(k == 0),
                        stop=(k == KT - 1),
                    )
                ysb = miscp.tile([128, ts_len], f32, name="ysb", tag="ysb")
                nc.vector.tensor_scalar_add(ysb[:], yps[:], bo_sb[:, m : m + 1])
                nc.gpsimd.dma_start(out=yt_d[128 * m : 128 * (m + 1), :], in_=ysb[:])

            # ---------------- new state output
            ns_sb = miscp.tile([HPC * DH, DH], f32, name="ns_sb", tag="nssb")
            nc.vector.tensor_add(ns_sb[:], s_acc[:], state_sb[:])
            nc.gpsimd.dma_start(out=ns_d[:], in_=ns_sb[:])

    nc.compile()
    return nc


def shard_inputs(state, x, w_qkv, b_qkv, w_out, b_out, mm_mode=MM_MODE,
                 num_cores=NCORES):
    """Host-side sharding/marshaling. Returns in_maps (one dict per core)."""
    if mm_mode == "bf16":
        import ml_dtypes
        mdt = ml_dtypes.bfloat16
    else:
        mdt = np.float32

    state = np.asarray(state, np.float32)
    x = np.asarray(x, np.float32)
    w_qkv = np.asarray(w_qkv, np.float32)
    b_qkv = np.asarray(b_qkv, np.float32)
    w_out = np.asarray(w_out, np.float32)
    b_out = np.asarray(b_out, np.float32)

    xt = np.ascontiguousarray(x.T).astype(mdt)          # (D, T)
    wo = w_out.astype(mdt)                              # (D, D)
    bo = b_out.reshape(KT, 128)                         # (8, 128)

    d_embd = x.shape[1]
    in_maps = []
    for core in range(num_cores):
        h0 = HPC * core
        cols = np.concatenate(
            [
                np.arange(g * d_embd + (h0 + hh) * DH, g * d_embd + (h0 + hh) * DH + DH)
                for g in range(3)
                for hh in range(HPC)
            ]
        )
        in_maps.append(
            {
                "xt": xt,
                "wqkv": np.ascontiguousarray(w_qkv[:, cols]).astype(mdt),
                "bqkv": np.ascontiguousarray(b_qkv[cols].reshape(3, HPC * DH)),
                "state0": np.ascontiguousarray(
                    state[h0 : h0 + HPC].reshape(HPC * DH, DH)
                ),
                "wout": wo,
                "bout": np.ascontiguousarray(bo),
            }
        )
    return in_maps


def assemble_outputs(results, t_len=T_FULL, num_cores=NCORES):
    ts_len = t_len // num_cores
    y = np.empty((t_len, D), np.float32)
    new_state = np.empty((H, DH, DH), np.float32)
    for core in range(num_cores):
        y[ts_len * core : ts_len * (core + 1), :] = results[core]["yt"].T
        new_state[HPC * core : HPC * (core + 1)] = results[core]["ns"].reshape(
            HPC, DH, DH
        )
    return new_state, y


_NC_CACHE = {}
LAST_RESULTS = None


def kernel(state, x, w_qkv, b_qkv, w_out, b_out):
    global LAST_RESULTS
    from concourse import bass_utils

    key = (MM_MODE,)
    if key not in _NC_CACHE:
        _NC_CACHE[key] = build_nc(T_FULL, MM_MODE, NCORES)
    nc = _NC_CACHE[key]

    in_maps = shard_inputs(state, x, w_qkv, b_qkv, w_out, b_out, MM_MODE, NCORES)
    res = bass_utils.run_bass_kernel_spmd(
        nc, in_maps, core_ids=list(range(NCORES))
    )
    LAST_RESULTS = res
    return assemble_outputs(res.results, T_FULL, NCORES)


# revision 24
# speedup vs baseline: 1.1565x; 1.0198x over previous
"""Distributed Trainium2 (Bass/Tile) kernel for causal *linear* attention
(unnormalized tril(q k^T) attention + state read/update) with qkv/out
projections, head-sharded over 8 NeuronCores.

Math identity used: with no softmax,
    out[t] = sum_{s<=t} (q_t . k_s) v_s + q_t @ state
           = q_t @ (state + sum_{s<=t} k_s v_s^T)
so attention is computed as a chunked scan (512-wide t-supers, 128-wide
s-blocks) instead of the O(T^2) dense form.

Distribution (8 cores): heads tensor-parallel (2 heads/core), lin_qkv
column-parallel, attention fully local per head; then one AllToAll turns the
head-sharded attention output into a sequence-sharded one, and each core runs
the out-projection for its own 512-row slice of the sequence (y computed
feature-major as yT = w_out^T @ out). Host only slices/concats/transposes.
"""

import os

import numpy as np

# ---------------------------------------------------------------- constants
T_FULL = 4096
D = 1024
H = 16
DH = 64
NCORES = 8
HPC = H // NCORES            # heads per core = 2
SUP = 512                    # t-super width
BLK = 128                    # s-block width
KT = D // 128                # k-tiles over embedding dim = 8
GW = HPC * DH                # per-group width of the qkv slice = 128

# matmul compute mode: "f32r" (fast fp32), "f32" (exact, 4x slower), "bf16"
MM_MODE = os.environ.get("KERNEL_MM_MODE", "bf16")


def build_nc(t_len=T_FULL, mm_mode=MM_MODE, num_cores=NCORES, sup=SUP):
    import concourse.bass as bass  # noqa: F401
    import concourse.mybir as mybir
    import concourse.tile as tile
    from concourse import bacc
    from concourse.masks import make_identity, make_upper_triangular

    f32 = mybir.dt.float32
    if mm_mode == "bf16":
        MDT = mybir.dt.bfloat16
    elif mm_mode == "f32r":
        # native float32r end-to-end: DMA keeps it, DVE writes perform the
        # fp32r rounding the BIR verifier demands of matmult inputs
        MDT = mybir.dt.float32r
    else:
        MDT = f32

    def mm(ap):
        return ap

    def mmf(ap):
        # fp32r matmults may not write PSUM at base partition != 0; for the
        # tiny S-update matmuls (N=64: f32 and f32r cost the same 4 cyc/row)
        # run in plain f32 instead
        return ap.bitcast(f32) if mm_mode == "f32r" else ap

    nsup = t_len // sup
    nblk = sup // BLK
    ts_len = t_len // num_cores   # per-core output sequence slice
    feat = num_cores * BLK        # gathered attention-feature dim (=D at 8 cores)
    # phase-1 supers map 1:1 onto a2a shards (t-chunks of the out-projection)
    assert ts_len == sup and nsup == num_cores

    nc = bacc.Bacc(
        "TRN2",
        target_bir_lowering=False,
        debug=False,
        enable_asserts=False,
        num_devices=num_cores,
    )

    xt_d = nc.dram_tensor("xt", [D, t_len], MDT, kind="ExternalInput")
    wqkv_d = nc.dram_tensor("wqkv", [D, 3 * GW], MDT, kind="ExternalInput")
    bqkv_d = nc.dram_tensor("bqkv", [3, GW], f32, kind="ExternalInput")
    st_d = nc.dram_tensor("state0", [HPC * DH, DH], f32, kind="ExternalInput")
    wout_d = nc.dram_tensor("wout", [feat, D], MDT, kind="ExternalInput")
    bout_d = nc.dram_tensor("bout", [KT, 128], f32, kind="ExternalInput")
    yt_d = nc.dram_tensor("yt", [D, ts_len], f32, kind="ExternalOutput")
    ns_d = nc.dram_tensor("ns", [HPC * DH, DH], f32, kind="ExternalOutput")

    with tile.TileContext(nc) as tc:
        with (
            tc.tile_pool(name="consts", bufs=1) as consts,
            tc.tile_pool(name="wpool", bufs=1) as wpool,
            tc.tile_pool(name="xtp", bufs=(4 if mm_mode == "bf16" else 3)) as xtp,
            tc.tile_pool(name="qkvp", bufs=2) as qkvp,
            tc.tile_pool(name="natp", bufs=2) as natp,
            tc.tile_pool(name="attp", bufs=4) as attp,
            tc.tile_pool(name="outp", bufs=2) as outp,
            tc.tile_pool(name="miscp", bufs=2) as miscp,
            tc.tile_pool(name="agp", bufs=1) as agp,
            tc.tile_pool(name="pp_ps", bufs=2, space="PSUM") as pp_ps,
            tc.tile_pool(name="at_ps", bufs=2, space="PSUM") as at_ps,
            tc.tile_pool(name="tr_ps", bufs=1, space="PSUM") as tr_ps,
            tc.tile_pool(name="ot_ps", bufs=1, space="PSUM") as ot_ps,
            tc.tile_pool(name="s_ps", bufs=1, space="PSUM") as s_ps,
            tc.tile_pool(name="dram", bufs=1, space="DRAM") as dram,
        ):
            # ---------------- constants / weights
            # PE transposes run in plain f32 when MDT is float32r (memset/
            # affine_select can't write fp32r, and transpose-mode is exact)
            TDT = f32 if mm_mode == "f32r" else MDT

            def td(ap):
                return ap.bitcast(TDT) if TDT is not MDT else ap

            # qkv weights + first x super first: they gate the first matmul
            wq_sb = wpool.tile([128, KT * 3 * GW], MDT, name="wq_sb")

            ident = consts.tile([128, 128], TDT, name="ident")
            make_identity(nc, ident[:])
            mask = consts.tile([BLK, sup], f32, name="mask")
            make_upper_triangular(nc, mask[:, :BLK], val=1.0, diag=True)
            if sup > BLK:
                nc.gpsimd.memset(mask[:, BLK:], 1.0)

            state_sb = consts.tile([HPC * DH, DH], f32, name="state_sb")
            bq_sb = consts.tile([GW, 3], f32, name="bq_sb")
            bo_sb = consts.tile([128, KT], f32, name="bo_sb")

            # a2a buffers: (num_cores * BLK, ts_len); shard j = t-chunk j,
            # feature rows (2 heads x 64) within the 128-row block.
            a2a_in = dram.tile([num_cores * BLK, ts_len], MDT, name="a2a_in")
            a2a_out = dram.tile([num_cores * BLK, ts_len], MDT, name="a2a_out")
            # last super's shard travels via a small AllGather instead (the
            # AllToAll is issued one super early to overlap the last super)
            ag2_in = dram.tile([BLK, ts_len], MDT, name="ag2_in")
            ag2_out = dram.tile(
                [num_cores * BLK, ts_len], MDT, name="ag2_out", addr_space="Shared"
            )
            zero_sb = consts.tile([BLK, ts_len], MDT, name="zero_sb")
            nc.gpsimd.memset(td(zero_sb[:]), 0.0)
            # a2a shard for the last core is dummy (real data goes via AG);
            # keep it finite
            nc.gpsimd.dma_start(
                out=a2a_in[BLK * (num_cores - 1) : BLK * num_cores, :], in_=zero_sb[:]
            )

            # running state accumulator (psum, fp32), heads stacked on partitions
            s_acc = s_ps.tile([HPC * DH, DH], f32, name="s_acc", tag="sacc")

            # ---------------- phase 1: qkv proj + chunked linear attention
            for tau in range(nsup):
                tsl = slice(sup * tau, sup * (tau + 1))
                xt_sb = xtp.tile([128, KT * sup], MDT, name="xt_sb", tag="xt")
                for k in range(KT):
                    if tau == 0:
                        # pairwise with xt k so proj MM k is gated by only
                        # 2(k+1) queued DMAs
                        nc.sync.dma_start(
                            out=wq_sb[:, 3 * GW * k : 3 * GW * (k + 1)],
                            in_=wqkv_d[128 * k : 128 * (k + 1), :],
                        )
                    nc.sync.dma_start(
                        out=xt_sb[:, sup * k : sup * (k + 1)],
                        in_=xt_d[128 * k : 128 * (k + 1), tsl],
                    )
                if tau == 0:
                    # small strided const loads ride behind the first super's
                    # bulk loads (needed a few us into super 0)
                    nc.sync.dma_start(out=state_sb[:], in_=st_d[:])
                    for g in range(3):
                        nc.sync.dma_start(
                            out=bq_sb[:, g : g + 1], in_=bqkv_d[g : g + 1, :]
                        )

                # qkvT projection, feature-major: group g in (q, k, v)
                grp_sb = []
                for g in range(3):
                    ps = pp_ps.tile([128, sup], f32, name=f"proj_ps{g}", tag="pp")
                    for k in range(KT):
                        nc.tensor.matmul(
                            out=ps[:],
                            lhsT=mm(wq_sb[:, 3 * GW * k + GW * g : 3 * GW * k + GW * (g + 1)]),
                            rhs=mm(xt_sb[:, sup * k : sup * (k + 1)]),
                            start=(k == 0),
                            stop=(k == KT - 1),
                        )
                    sb = qkvp.tile([128, sup], MDT, name=f"g{g}T_sb", tag=f"g{g}T")
                    nc.vector.tensor_scalar_add(sb[:], ps[:], bq_sb[:, g : g + 1])
                    grp_sb.append(sb)
                qT_sb, kT_sb, vT_sb = grp_sb

                # transpose k, v to t-major (s on partitions) via PE
                k_nat = natp.tile([128, sup], MDT, name="k_nat", tag="knat")
                v_nat = natp.tile([128, sup], MDT, name="v_nat", tag="vnat")
                for srct, dst in ((kT_sb, k_nat), (vT_sb, v_nat)):
                    for j in range(nblk):
                        bsl = slice(BLK * j, BLK * (j + 1))
                        tp = tr_ps.tile([128, BLK], TDT, name="tp_ps", tag="tp")
                        nc.tensor.transpose(tp[:], td(srct[:, bsl]), ident[:])
                        nc.vector.tensor_copy(dst[:, bsl], tp[:])

                # snapshot of state for this super (state_init + prefix sums)
                s_sb = miscp.tile([HPC * DH, DH], MDT, name="s_sb", tag="ssb")
                for h in range(HPC):
                    hsl = slice(DH * h, DH * (h + 1))
                    if tau == 0:
                        nc.vector.tensor_copy(s_sb[hsl, :], state_sb[hsl, :])
                    else:
                        nc.vector.tensor_add(s_sb[hsl, :], s_acc[hsl, :], state_sb[hsl, :])

                out_ps_h = [
                    ot_ps.tile([DH, sup], f32, name=f"out_ps{h}", tag=f"ops{h}")
                    for h in range(HPC)
                ]
                # the two heads' K=64 matmuls sit at row-group bases 0/64;
                # emitting each pair back-to-back lets the PE run them
                # concurrently in disjoint 32x32 subarrays
                for h in range(HPC):
                    hsl = slice(DH * h, DH * (h + 1))
                    # state contribution covers all t of the super
                    nc.tensor.matmul(
                        out=out_ps_h[h][:, :],
                        lhsT=mm(s_sb[hsl, :]),
                        rhs=mm(qT_sb[hsl, :]),
                        start=True,
                        stop=False,
                    )
                for j in range(nblk):
                    bsl = slice(BLK * j, BLK * (j + 1))
                    used = sup - BLK * j           # out columns [BLK*j, sup)
                    if j < nblk - 1 or nblk == 1 or mm_mode != "f32r":
                        width, rstart, dg0 = used, BLK * j, 0
                    else:
                        # widen N to 256 so f32r stays on the 1-cyc path
                        width, rstart, dg0 = 2 * BLK, sup - 2 * BLK, BLK
                    aps_h, asb_h = [], []
                    for h in range(HPC):
                        hsl = slice(DH * h, DH * (h + 1))
                        aps = at_ps.tile([128, sup], f32, name=f"aps{h}", tag="aps")
                        nc.tensor.matmul(
                            out=aps[:, :width],
                            lhsT=mm(kT_sb[hsl, bsl]),
                            rhs=mm(qT_sb[hsl, rstart:sup]),
                            start=True,
                            stop=True,
                        )
                        aps_h.append(aps)
                    for h in range(HPC):
                        asb = attp.tile([128, sup], MDT, name=f"asb{h}", tag="asb")
                        # one op: triu mask on the diagonal 128 block, pass-
                        # through (x1.0) beyond it
                        nc.vector.tensor_mul(
                            asb[:, dg0 : dg0 + used],
                            aps_h[h][:, dg0 : dg0 + used],
                            mask[:, :used],
                        )
                        asb_h.append(asb)
                    for h in range(HPC):
                        nc.tensor.matmul(
                            out=out_ps_h[h][:, BLK * j : sup],
                            lhsT=mm(v_nat[:, BLK * j + DH * h : BLK * j + DH * (h + 1)]),
                            rhs=mm(asb_h[h][:, dg0 : dg0 + used]),
                            start=False,
                            stop=(j == nblk - 1),
                        )
                    for h in range(HPC):
                        hsl = slice(DH * h, DH * (h + 1))
                        # state update: S += k_blk^T v_blk (one psum group over
                        # the whole scan; reads interleave, so skip group check)
                        nc.tensor.matmul(
                            out=s_acc[hsl, :],
                            lhsT=mmf(k_nat[:, BLK * j + DH * h : BLK * j + DH * (h + 1)]),
                            rhs=mmf(v_nat[:, BLK * j + DH * h : BLK * j + DH * (h + 1)]),
                            start=(tau == 0 and j == 0),
                            stop=(tau == nsup - 1 and j == nblk - 1),
                            skip_group_check=True,
                        )

                if True:
                    for h in range(HPC):
                        o_sb = outp.tile([DH, sup], MDT, name=f"o_sb{h}", tag=f"osb{h}")
                        nc.vector.tensor_copy(o_sb[:], out_ps_h[h][:])
                        if tau < nsup - 1:
                            dst = a2a_in[BLK * tau + DH * h : BLK * tau + DH * (h + 1), :]
                        else:
                            dst = ag2_in[DH * h : DH * (h + 1), :]
                        nc.gpsimd.dma_start(out=dst, in_=o_sb[:])
                if tau == nsup - 2:
                    # all destinations except the last core are final: launch
                    # the AllToAll now so it overlaps the last super's compute
                    nc.gpsimd.collective_compute(
                        "AllToAll",
                        mybir.AluOpType.bypass,
                        replica_groups=[list(range(num_cores))],
                        ins=[a2a_in.opt()],
                        outs=[a2a_out.opt()],
                    )

            # out-proj weights: only needed after the collective; loading here
            # overlaps the DMAs with late-phase-1 compute
            wo_sb = wpool.tile([128, num_cores * D], MDT, name="wo_sb")
            for k in range(num_cores):
                nc.sync.dma_start(
                    out=wo_sb[:, D * k : D * (k + 1)],
                    in_=wout_d[128 * k : 128 * (k + 1), :],
                )
            for g in range(KT):
                nc.sync.dma_start(out=bo_sb[:, g : g + 1], in_=bout_d[g : g + 1, :])

            # ---------------- phase 2: gather last shard + out-projection
            nc.gpsimd.collective_compute(
                "AllGather",
                mybir.AluOpType.bypass,
                replica_groups=[list(range(num_cores))],
                ins=[ag2_in.opt()],
                outs=[ag2_out.opt()],
            )
            ag_sb = agp.tile([128, num_cores * ts_len], MDT, name="ag_sb")
            for k in range(num_cores):
                nc.sync.dma_start(
                    out=ag_sb[:, ts_len * k : ts_len * (k + 1)],
                    in_=a2a_out[128 * k : 128 * (k + 1), :],
                )
            # on the last core the AllToAll result is dummy - overwrite from
            # the AllGather (predicated DMAs; skipped elsewhere)
            is_last = nc.sync.partition_id() == (num_cores - 1)
            for k in range(num_cores):
                nc.sync.dma_start(
                    out=ag_sb[:, ts_len * k : ts_len * (k + 1)],
                    in_=ag2_out[128 * k : 128 * (k + 1), :],
                    cond=is_last,
                )
            for m in range(KT):
                yps = pp_ps.tile([128, ts_len], f32, name="y_ps", tag="pp")
                for k in range(num_cores):
                    nc.tensor.matmul(
                        out=yps[:],
                        lhsT=mm(wo_sb[:, D * k + 128 * m : D * k + 128 * (m + 1)]),
                        rhs=mm(ag_sb[:, ts_len * k : ts_len * (k + 1)]),
                        start=(k == 0),
                        stop=(k == KT - 1),
                    )
                ysb = miscp.tile([128, ts_len], f32, name="ysb", tag="ysb")
                nc.vector.tensor_scalar_add(ysb[:], yps[:], bo_sb[:, m : m + 1])
                nc.gpsimd.dma_start(out=yt_d[128 * m : 128 * (m + 1), :], in_=ysb[:])

            # ---------------- new state output
            ns_sb = miscp.tile([HPC * DH, DH], f32, name="ns_sb", tag="nssb")
            nc.vector.tensor_add(ns_sb[:], s_acc[:], state_sb[:])
            nc.gpsimd.dma_start(out=ns_d[:], in_=ns_sb[:])

    nc.compile()
    return nc


def shard_inputs(state, x, w_qkv, b_qkv, w_out, b_out, mm_mode=MM_MODE,
                 num_cores=NCORES):
    """Host-side sharding/marshaling. Returns in_maps (one dict per core)."""
    if mm_mode == "bf16":
        import ml_dtypes
        mdt = ml_dtypes.bfloat16
    else:
        mdt = np.float32

    state = np.asarray(state, np.float32)
    x = np.asarray(x, np.float32)
    w_qkv = np.asarray(w_qkv, np.float32)
    b_qkv = np.asarray(b_qkv, np.float32)
    w_out = np.asarray(w_out, np.float32)
    b_out = np.asarray(b_out, np.float32)

    xt = np.ascontiguousarray(x.T).astype(mdt)          # (D, T)
    wo = w_out.astype(mdt)                              # (D, D)
    bo = b_out.reshape(KT, 128)                         # (8, 128)

    d_embd = x.shape[1]
    in_maps = []
    for core in range(num_cores):
        h0 = HPC * core
        cols = np.concatenate(
            [
                np.arange(g * d_embd + (h0 + hh) * DH, g * d_embd + (h0 + hh) * DH + DH)
                for g in range(3)
                for hh in range(HPC)
            ]
        )
        in_maps.append(
            {
                "xt": xt,
                "wqkv": np.ascontiguousarray(w_qkv[:, cols]).astype(mdt),
                "bqkv": np.ascontiguousarray(b_qkv[cols].reshape(3, HPC * DH)),
                "state0": np.ascontiguousarray(
                    state[h0 : h0 + HPC].reshape(HPC * DH, DH)
                ),
                "wout": wo,
                "bout": np.ascontiguousarray(bo),
            }
        )
    return in_maps


def assemble_outputs(results, t_len=T_FULL, num_cores=NCORES):
    ts_len = t_len // num_cores
    y = np.empty((t_len, D), np.float32)
    new_state = np.empty((H, DH, DH), np.float32)
    for core in range(num_cores):
        y[ts_len * core : ts_len * (core + 1), :] = results[core]["yt"].T
        new_state[HPC * core : HPC * (core + 1)] = results[core]["ns"].reshape(
            HPC, DH, DH
        )
    return new_state, y


_NC_CACHE = {}
LAST_RESULTS = None


def kernel(state, x, w_qkv, b_qkv, w_out, b_out):
    global LAST_RESULTS
    from concourse import bass_utils

    key = (MM_MODE,)
    if key not in _NC_CACHE:
        _NC_CACHE[key] = build_nc(T_FULL, MM_MODE, NCORES)
    nc = _NC_CACHE[key]

    in_maps = shard_inputs(state, x, w_qkv, b_qkv, w_out, b_out, MM_MODE, NCORES)
    res = bass_utils.run_bass_kernel_spmd(
        nc, in_maps, core_ids=list(range(NCORES))
    )
    LAST_RESULTS = res
    return assemble_outputs(res.results, T_FULL, NCORES)
